# revision 1
# baseline (speedup 1.0000x reference)
"""Trainium2 Bass kernel for a 2-layer multi-head GAT (nn_MultiHeadGATLayer).

Architecture recap (hardcoded, matches the reference):
  N=16384 nodes, D=512 feats, E=540672 edges (32 random in-edges/node + self loop),
  layer 1: 8 heads x 64 dims with per-head attention + elu, concat;
  layer 2: single 512-dim GAT head over the concat + elu; residual with input.

Distribution: nodes are sharded across 8 NeuronCores (destination sharding).
Each core computes z = x @ W for its node shard, the shards are AllGathered
into a per-core z-table in HBM, and each core then runs the edge phase for its
own destination nodes: indirect-DMA gathers of z[src] rows (640-elem bf16 rows
with the per-head source attention logits embedded at cols 512:520), softmax
over incoming edges (no max-subtraction: logits are bounded by construction,
self-loops keep denominators positive; padding slots point at a dummy row
whose embedded logit is -3e38 so exp()=0), and a weighted sum done as
DVE multiply (att broadcast over 64-wide head blocks in 2x mode via duplicated
attention pairs) + TensorE identity-matmul accumulation into PSUM.

Host side does layout only: degree-sorted node permutation, padded gather
index construction (int16, 16-partition wrap), weight reshapes/transposes.
"""
import os
import sys

sys.path.insert(0, "/opt/trn_rl_repo")

import numpy as np
import ml_dtypes

import concourse.bacc as bacc
import concourse.mybir as mybir
from concourse.tile import TileContext
from concourse.bass_utils import run_bass_kernel_spmd
from concourse.library_config import mlp

F32 = mybir.dt.float32
BF16 = mybir.dt.bfloat16
I16 = mybir.dt.int16

B, S, D = 64, 256, 512
H, DO = 8, 64
ALPHA = 0.2
N = B * S
DEG = 32
E = N * DEG + N
NCORES = 8
P = 128
SHN = N // NCORES          # nodes per core (2048)
NT = SHN // P              # node tiles per core (16)
KG = 8                     # slots per gather call
ROWW = 640                 # z-table row width (512 z + 8 es + pad), bf16
DUMMY = N                  # dummy row index for padding slots
NEG = -3.0e38

_cache = {}


def _build_host(src, dst):
    """Host-side layout: permutation, per-core padded gather indices."""
    deg = np.bincount(dst, minlength=N)
    order = np.argsort(-deg, kind="stable")          # nodes by degree desc
    # deal round-robin so all cores see the same degree profile
    core_of = np.empty(N, np.int32)
    pos_of = np.empty(N, np.int32)
    for c in range(NCORES):
        nodes_c = order[c::NCORES]                   # 2048 nodes, deg-sorted desc
        core_of[nodes_c] = c
        pos_of[nodes_c] = np.arange(SHN)
    nodes = [order[c::NCORES] for c in range(NCORES)]
    tabpos = (core_of.astype(np.int64) * SHN + pos_of).astype(np.int32)

    # in-edges per node: sort edges by dst
    eorder = np.argsort(dst, kind="stable")
    src_s = src[eorder]
    cum = np.zeros(N + 1, np.int64)
    np.cumsum(deg, out=cum[1:])

    # common slot schedule: K_sched[j] = max over cores of tile max degree
    K_sched = np.zeros(NT, np.int64)
    for c in range(NCORES):
        dg = deg[nodes[c]]
        for j in range(NT):
            K_sched[j] = max(K_sched[j], dg[j * P:(j + 1) * P].max())
    K_sched = ((K_sched + KG - 1) // KG) * KG
    totK = int(K_sched.sum())

    # padded source table per core: [NT, K_j, P] slot-major within tile
    idx_cores = []
    for c in range(NCORES):
        blocks = []
        nds = nodes[c]
        for j in range(NT):
            nj = nds[j * P:(j + 1) * P]
            Kj = int(K_sched[j])
            pad = np.full((P, Kj), DUMMY, np.int32)
            for i, n in enumerate(nj):
                d0 = int(deg[n])
                pad[i, :d0] = tabpos[src_s[cum[n]:cum[n] + d0]]
            blocks.append(pad.T.reshape(-1))         # slot-major: [Kj, P]
        flat = np.concatenate(blocks).astype(np.int32)   # [totK*P]
        # int16 wrap layout: idx i -> partition i%16, col i//16 ; replicate 8x
        assert flat.max() <= 32767
        w = flat.reshape(-1, 16).T                   # [16, totK*P/16]
        idx_cores.append(np.tile(w, (8, 1)).astype(np.int16))
    return nodes, K_sched, totK, idx_cores


def _build_program(K_sched, totK):
    nc = bacc.Bacc("TRN2", target_bir_lowering=False, debug=False,
                   num_devices=NCORES)
    KT = [int(k) for k in K_sched]
    IDXW = totK * P // 16

    xT_in = nc.dram_tensor("xT", [D, SHN], F32, kind="ExternalInput")
    x_in = nc.dram_tensor("x", [SHN, D], F32, kind="ExternalInput")
    w1_in = nc.dram_tensor("w1", [D, D], F32, kind="ExternalInput")       # W1cat
    w1t_in = nc.dram_tensor("w1t", [D, D], F32, kind="ExternalInput")     # W1cat.T
    a1_in = nc.dram_tensor("a1", [D, 16], F32, kind="ExternalInput")      # [A1s | A1d]
    w2_in = nc.dram_tensor("w2", [D, D], F32, kind="ExternalInput")       # Wout
    w2t_in = nc.dram_tensor("w2t", [D, D], F32, kind="ExternalInput")     # Wout.T
    a2_in = nc.dram_tensor("a2", [D, 2], F32, kind="ExternalInput")       # [aout_s | aout_d]
    id_in = nc.dram_tensor("ident", [P, P], F32, kind="ExternalInput")
    idx1_in = nc.dram_tensor("idx1", [P, IDXW], I16, kind="ExternalInput")
    idx2_in = nc.dram_tensor("idx2", [P, IDXW], I16, kind="ExternalInput")
    out = nc.dram_tensor("out", [SHN, D], F32, kind="ExternalOutput")

    zloc1 = nc.dram_tensor("zloc1", [SHN, ROWW], BF16, kind="Internal")
    zloc2 = nc.dram_tensor("zloc2", [SHN, ROWW], BF16, kind="Internal")
    ztab1 = nc.dram_tensor("ztab1", [N + P, ROWW], BF16, kind="Internal",
                           addr_space="Shared")
    ztab2 = nc.dram_tensor("ztab2", [N + P, ROWW], BF16, kind="Internal",
                           addr_space="Shared")

    with TileContext(nc) as tc:
        with tc.tile_pool(name="const", bufs=1) as cpool, \
             tc.tile_pool(name="work", bufs=2) as wpool, \
             tc.tile_pool(name="gat", bufs=2) as gpool, \
             tc.tile_pool(name="wg", bufs=3) as wgpool, \
             tc.tile_pool(name="pz", bufs=2, space="PSUM") as pzpool, \
             tc.tile_pool(name="pe", bufs=2, space="PSUM") as pepool, \
             tc.tile_pool(name="pt", bufs=2, space="PSUM") as ptpool:

            nc.gpsimd.load_library(mlp)

            # ---------- setup: constants, weights ----------
            identf = cpool.tile([P, P], F32)
            identb = cpool.tile([P, P], BF16)
            nc.sync.dma_start(identf[:], id_in[:])
            nc.vector.tensor_copy(identb[:], identf[:])

            idx1 = cpool.tile([P, IDXW], I16)
            idx2 = cpool.tile([P, IDXW], I16)
            nc.sync.dma_start(idx1[:], idx1_in[:])
            nc.sync.dma_start(idx2[:], idx2_in[:])

            # weights: rhs chunks [128, 512] bf16 (4 per layer) + va cols
            w1b, w2b, w1tf, w2tf = [], [], [], []
            for cidx in range(4):
                wf = wpool.tile([P, D], F32, tag="wload")
                nc.sync.dma_start(wf[:], w1_in[cidx * P:(cidx + 1) * P, :])
                wb = cpool.tile([P, D], BF16, tag=f"w1b{cidx}")
                nc.vector.tensor_copy(wb[:], wf[:])
                w1b.append(wb)
                wf2 = wpool.tile([P, D], F32, tag="wload")
                nc.sync.dma_start(wf2[:], w2_in[cidx * P:(cidx + 1) * P, :])
                wb2 = cpool.tile([P, D], BF16, tag=f"w2b{cidx}")
                nc.vector.tensor_copy(wb2[:], wf2[:])
                w2b.append(wb2)
                t1 = cpool.tile([P, D], F32, tag=f"w1t{cidx}")
                nc.sync.dma_start(t1[:], w1t_in[cidx * P:(cidx + 1) * P, :])
                w1tf.append(t1)
                t2 = cpool.tile([P, D], F32, tag=f"w2t{cidx}")
                nc.sync.dma_start(t2[:], w2t_in[cidx * P:(cidx + 1) * P, :])
                w2tf.append(t2)

            a1f = [cpool.tile([P, 16], F32, tag=f"a1f{c}", name=f"a1f{c}") for c in range(4)]
            a2f = [cpool.tile([P, 2], F32, tag=f"a2f{c}", name=f"a2f{c}") for c in range(4)]
            for cidx in range(4):
                nc.sync.dma_start(a1f[cidx][:], a1_in[cidx * P:(cidx + 1) * P, :])
                nc.sync.dma_start(a2f[cidx][:], a2_in[cidx * P:(cidx + 1) * P, :])

            # va1[d,16] = W1cat.T-chunks.T @ A1 blocks ; va2[d,2]
            va1 = []
            va2 = []
            for dchunk in range(4):
                pv = pzpool.tile([P, 16], F32, tag="pzA")
                for fc in range(4):
                    nc.tensor.matmul(
                        pv[:], w1tf[fc][:, dchunk * P:(dchunk + 1) * P],
                        a1f[fc][:],
                        start=(fc == 0), stop=(fc == 3))
                vb = cpool.tile([P, 16], BF16, tag=f"va1{dchunk}")
                nc.vector.tensor_copy(vb[:], pv[:])
                va1.append(vb)
                pv2 = pzpool.tile([P, 2], F32, tag="pzB", bufs=1)
                for fc in range(4):
                    nc.tensor.matmul(
                        pv2[:], w2tf[fc][:, dchunk * P:(dchunk + 1) * P],
                        a2f[fc][:],
                        start=(fc == 0), stop=(fc == 3))
                vb2 = cpool.tile([P, 2], BF16, tag=f"va2{dchunk}")
                nc.vector.tensor_copy(vb2[:], pv2[:])
                va2.append(vb2)

            # dummy row (padding target): zeros except es cols = NEG
            drow = cpool.tile([1, ROWW], BF16)
            nc.vector.memset(drow[:], 0.0)
            nc.vector.memset(drow[:, 512:520], NEG)
            for ztab in (ztab1, ztab2):
                nc.sync.dma_start(ztab[N:N + 1, :], drow[:])

            ed1 = cpool.tile([P, NT, H], F32)
            ed2 = cpool.tile([P, NT, 1], F32)
            hcTb = [cpool.tile([P, SHN], BF16, tag=f"hcT{c}", name=f"hcT{c}") for c in range(4)]

            # ---------- phase 1: z1 shard ----------
            xTb = [cpool.tile([P, SHN], BF16, tag=f"xT{c}", name=f"xT{c}") for c in range(4)]
            for cidx in range(4):
                xf = wpool.tile([P, SHN], F32, tag="xload")
                nc.sync.dma_start(xf[:], xT_in[cidx * P:(cidx + 1) * P, :])
                nc.vector.tensor_copy(xTb[cidx][:], xf[:])

            def z_phase(lhs_blocks, wb, va, zloc, ed_store, ncols):
                for nt in range(NT):
                    pa = pzpool.tile([P, D], F32, tag="pzA")
                    pb = pzpool.tile([P, 16], F32, tag="pzB", bufs=1)
                    for cidx in range(4):
                        lb = lhs_blocks[cidx][:, nt * P:(nt + 1) * P]
                        nc.tensor.matmul(pa[:], lb, wb[cidx][:],
                                         start=(cidx == 0), stop=(cidx == 3))
                    for cidx in range(4):
                        lb = lhs_blocks[cidx][:, nt * P:(nt + 1) * P]
                        nc.tensor.matmul(pb[:, 0:2 * ncols], lb, va[cidx][:],
                                         start=(cidx == 0), stop=(cidx == 3))
                    zrow = wpool.tile([P, ROWW], BF16, tag="zrow")
                    nc.vector.tensor_copy(zrow[:, 0:D], pa[:])
                    nc.vector.tensor_copy(zrow[:, D:D + ncols], pb[:, 0:ncols])
                    nc.vector.memset(zrow[:, D + ncols:ROWW], 0.0)
                    nc.vector.tensor_copy(
                        ed_store[:, nt, :],
                        pb[:, ncols:2 * ncols])
                    nc.sync.dma_start(zloc[nt * P:(nt + 1) * P, :], zrow[:])

            z_phase([xTb[c][:] for c in range(4)], w1b, va1, zloc1, ed1, H)

            nc.gpsimd.collective_compute(
                "AllGather", mybir.AluOpType.bypass,
                replica_groups=[list(range(NCORES))],
                ins=[zloc1[:]], outs=[ztab1[0:N, :]])


            # ---------- edge phase (shared for both layers) ----------
            def edge_phase(ztab, idx, nheads, ed_store, out_cb):
                rep = D // nheads            # 64 or 512
                idx_off = 0
                for nt in range(NT):
                    Kj = KT[nt]
                    nkg = Kj // KG
                    po = pepool.tile([P, D], F32, tag="pout")
                    den = wpool.tile([P, nheads], F32, tag="den")
                    for kg in range(nkg):
                        g = gpool.tile([P, KG, ROWW], BF16, tag="G")
                        nidx = P * KG
                        nc.gpsimd.dma_gather(
                            g[:], ztab[:], idx[:, idx_off:idx_off + nidx // 16],
                            nidx, nidx, ROWW)
                        idx_off += nidx // 16
                        # attention logits: s = es_gather + ed_local (dup'd pairs)
                        sd = wpool.tile([P, KG, nheads, 2], F32, tag="sd")
                        es_v = g[:, :, D:D + nheads].unsqueeze(3) \
                            .broadcast_to([P, KG, nheads, 2])
                        ed_v = ed_store[:, nt, :].unsqueeze(1).unsqueeze(3) \
                            .broadcast_to([P, KG, nheads, 2])
                        nc.vector.tensor_tensor(sd[:], es_v, ed_v,
                                                mybir.AluOpType.add)
                        ud = wpool.tile([P, KG, nheads, 2], F32, tag="ud")
                        nc.vector.tensor_scalar_mul(ud[:], sd[:], ALPHA)
                        nc.vector.tensor_tensor(sd[:], sd[:], ud[:],
                                                mybir.AluOpType.max)
                        ad = wpool.tile([P, KG, nheads, 2], BF16, tag="ad")
                        nc.scalar.activation(ad[:], sd[:],
                                             mybir.ActivationFunctionType.Exp)
                        dpart = wpool.tile([P, nheads], F32, tag="dpart")
                        nc.vector.tensor_reduce(
                            dpart[:], ad[:].rearrange("p k h t -> p h k t"),
                            mybir.AxisListType.XY, mybir.AluOpType.add)
                        if kg == 0:
                            nc.vector.tensor_copy(den[:], dpart[:])
                        else:
                            nc.vector.tensor_tensor(den[:], den[:], dpart[:],
                                                    mybir.AluOpType.add)
                        for k in range(KG):
                            wg = wgpool.tile([P, D], BF16, tag="wg")
                            g_v = g[:, k, 0:D].rearrange(
                                "p (h r t) -> p h r t", h=nheads, r=rep // 2, t=2)
                            a_v = ad[:, k, :, :].unsqueeze(2) \
                                .broadcast_to([P, nheads, rep // 2, 2])
                            w_v = wg[:].rearrange(
                                "p (h r t) -> p h r t", h=nheads, r=rep // 2, t=2)
                            nc.vector.tensor_tensor(w_v, g_v, a_v,
                                                    mybir.AluOpType.mult)
                            kk = kg * KG + k
                            nc.tensor.matmul(po[:], identb[:], wg[:],
                                             start=(kk == 0), stop=(kk == Kj - 1))
                    # normalize (x2 compensates the dup'd den) and activation
                    rcp = wpool.tile([P, nheads], F32, tag="rcp")
                    nc.vector.reciprocal(rcp[:], den[:])
                    t1 = wpool.tile([P, D], F32, tag="t1")
                    r_v = rcp[:].unsqueeze(2).broadcast_to([P, nheads, rep])
                    t_v = t1[:].rearrange("p (h r) -> p h r", h=nheads, r=rep)
                    nc.vector.scalar_tensor_tensor(
                        t_v, po[:].rearrange("p (h r) -> p h r", h=nheads, r=rep),
                        2.0, r_v, mybir.AluOpType.mult, mybir.AluOpType.mult)
                    out_cb(nt, t1)

            # layer-1 per-tile epilogue: elu -> bf16 -> transpose into hcTb
            def l1_out(nt, t1):
                # elu via ScalarE: em=relu(-t1); ex=exp(-em); pos=relu(t1)
                em = wpool.tile([P, D], F32, tag="em")
                nc.scalar.activation(em[:], t1[:],
                                     mybir.ActivationFunctionType.Relu,
                                     scale=-1.0)
                ex = wpool.tile([P, D], F32, tag="ex")
                nc.scalar.activation(ex[:], em[:],
                                     mybir.ActivationFunctionType.Exp,
                                     scale=-1.0)
                pos = wpool.tile([P, D], F32, tag="pos")
                nc.scalar.activation(pos[:], t1[:],
                                     mybir.ActivationFunctionType.Relu)
                hc = wpool.tile([P, D], BF16, tag="hc")
                nc.vector.scalar_tensor_tensor(
                    hc[:], ex[:], -1.0, pos[:],
                    mybir.AluOpType.add, mybir.AluOpType.add)
                for cidx in range(4):
                    pt = ptpool.tile([P, P], BF16, tag="ptr")
                    nc.tensor.transpose(pt[:], hc[:, cidx * P:(cidx + 1) * P],
                                        identb[:])
                    nc.vector.tensor_copy(
                        hcTb[cidx][:, nt * P:(nt + 1) * P], pt[:])

            edge_phase(ztab1, idx1, H, ed1, l1_out)

            # ---------- phase 3: z2 shard + AG ----------
            z_phase([hcTb[c][:] for c in range(4)], w2b, va2, zloc2, ed2, 1)
            nc.gpsimd.collective_compute(
                "AllGather", mybir.AluOpType.bypass,
                replica_groups=[list(range(NCORES))],
                ins=[zloc2[:]], outs=[ztab2[0:N, :]])

            # ---------- phase 4: L2 edge + residual ----------
            def l2_out(nt, t1):
                em = wpool.tile([P, D], F32, tag="em")
                nc.scalar.activation(em[:], t1[:],
                                     mybir.ActivationFunctionType.Relu,
                                     scale=-1.0)
                ex = wpool.tile([P, D], F32, tag="ex")
                nc.scalar.activation(ex[:], em[:],
                                     mybir.ActivationFunctionType.Exp,
                                     scale=-1.0)
                pos = wpool.tile([P, D], F32, tag="pos")
                nc.scalar.activation(pos[:], t1[:],
                                     mybir.ActivationFunctionType.Relu)
                el = wpool.tile([P, D], F32, tag="el")
                nc.vector.scalar_tensor_tensor(
                    el[:], ex[:], -1.0, pos[:],
                    mybir.AluOpType.add, mybir.AluOpType.add)
                xr = wpool.tile([P, D], F32, tag="xr")
                nc.sync.dma_start(xr[:], x_in[nt * P:(nt + 1) * P, :])
                ot = wpool.tile([P, D], F32, tag="ot")
                nc.vector.tensor_tensor(ot[:], el[:], xr[:],
                                        mybir.AluOpType.add)
                nc.sync.dma_start(out[nt * P:(nt + 1) * P, :], ot[:])

            edge_phase(ztab2, idx2, 1, ed2, l2_out)

    nc.compile()
    return nc


def kernel(h, W1, a1, Wout, aout, src, dst):
    h = np.asarray(h, np.float32)
    W1 = np.asarray(W1, np.float32)
    a1 = np.asarray(a1, np.float32)
    Wout = np.asarray(Wout, np.float32)
    aout = np.asarray(aout, np.float32)
    src = np.asarray(src, np.int32)
    dst = np.asarray(dst, np.int32)

    x = h.reshape(N, D)
    nodes, K_sched, totK, idx_cores = _build_host(src, dst)

    key = (tuple(int(k) for k in K_sched), totK)
    if key not in _cache:
        _cache[key] = _build_program(K_sched, totK)
    nc = _cache[key]

    # weight layouts
    W1cat = np.ascontiguousarray(W1.transpose(1, 0, 2).reshape(D, D))
    A1 = np.zeros((D, 16), np.float32)
    for hh in range(H):
        A1[hh * DO:(hh + 1) * DO, hh] = a1[hh, :DO]
        A1[hh * DO:(hh + 1) * DO, 8 + hh] = a1[hh, DO:]
    A2 = np.stack([aout[:D], aout[D:]], axis=1).astype(np.float32)
    ident = np.eye(P, dtype=np.float32)

    in_maps = []
    for c in range(NCORES):
        xs = np.ascontiguousarray(x[nodes[c]])
        in_maps.append({
            "xT": np.ascontiguousarray(xs.T),
            "x": xs,
            "w1": W1cat,
            "w1t": np.ascontiguousarray(W1cat.T),
            "a1": A1,
            "w2": Wout,
            "w2t": np.ascontiguousarray(Wout.T),
            "a2": A2,
            "ident": ident,
            "idx1": idx_cores[c],
            "idx2": idx_cores[c],
        })

    trace = bool(int(os.environ.get("GAT_TRACE", "0")))
    res = run_bass_kernel_spmd(nc, in_maps, core_ids=list(range(NCORES)),
                               trace=trace)
    if trace:
        print("HW exec time:", res.exec_time_ns, "ns")
        print("trace:", res.instructions_and_trace[1]
              if res.instructions_and_trace else None)
    outf = np.zeros((N, D), np.float32)
    for c in range(NCORES):
        outf[nodes[c]] = res.results[c]["out"]
    return outf.reshape(B, S, D)

